# revision 18
# baseline (speedup 1.0000x reference)
"""Trainium2 Bass kernel for nn_MLPRepairModule.

Math (B=8, Q=1, T=2048, H=512, V=32000):
  w1q, w1t = w1[:, :H], w1[:, H:]
  q_proj[b,k]   = sum_h input_embeds[b,0,h] * w1q[k,h]            (tiny -> host)
  qb1[b,k]      = q_proj[b,k] + b1[k]                             (host)
  d_proj[v,k]   = sum_h decoder_weight[v,h] * w1t[k,h]            (PE)
  t_proj[b,t,k] = sum_h target_embeds[b,t,h] * w1t[k,h]           (PE)
  dec_logits[b,v] = sum_k w2[k] * relu(d_proj[v,k] + qb1[b,k])    (DVE relu + PE matvec)
  rep_logits[b,t] = sum_k w2[k] * relu(t_proj[b,t,k] + qb1[b,k])
  out = [dec_logits, mask*rep_logits - 1000*(1-mask)]             (mask on host)

Sharding: V and T split across 8 cores (each core: 4000 vocab rows +
256 target positions, all 8 batch rows). MLP weights replicated.

Device layout: k on partitions ([k, v] tiles) so the per-batch bias add +
relu is a single DVE tensor_scalar (bf16 4x mode) with a per-partition
scalar, and the k-reduction is a PE matvec using 4-way column tiling
(tile_position col groups, one batch row per 32-col group). The vocab
axis is zero-padded to 4096 on the host so every tile boundary is
512-aligned (PSUM bank width).
"""

import os
import sys

if "/opt/trn_rl_repo" not in sys.path:
    sys.path.insert(0, "/opt/trn_rl_repo")

import ml_dtypes
import numpy as np

import concourse.bass as bass
from concourse import bacc
import concourse.mybir as mybir
import concourse.tile as tile
from concourse.bass_utils import run_bass_kernel_spmd

H = 512
B = 8
V = 32000
T = 2048
NCORES = 8
VC = V // NCORES  # 4000 vocab rows per core
VCP = 4096  # padded vocab rows per core (512-aligned)
TCC = T // NCORES  # 256 target positions per core
BT = B * TCC  # 2048 (b,t) columns per core
KC = H // 128  # 4 contraction chunks

BF16 = mybir.dt.bfloat16
F8 = mybir.dt.float8e4
F32 = mybir.dt.float32
AOP = mybir.AluOpType
BF16NP = ml_dtypes.bfloat16
F8NP = mybir.dt.np(mybir.dt.float8e4)
FP8_SCALE = 16.0

CW = 512  # matmul/psum chunk width
VSL = 1024  # relu slab width / projection-copy width (one ACT producer)
NSL = VCP // VSL  # 4

_cache: dict = {}
last_results = None


def _build_nc(n_reps: int = 1):
    nc = bacc.Bacc("TRN2", target_bir_lowering=False)

    decT8 = nc.dram_tensor("decT8", [2, 128, 2, VCP], F8, kind="ExternalInput")
    w1tT8 = nc.dram_tensor("w1tT8", [2, 128, 2, H], F8, kind="ExternalInput")
    tgtT = nc.dram_tensor("tgtT", [H, BT], BF16, kind="ExternalInput")
    w1tT = nc.dram_tensor("w1tT", [H, H], BF16, kind="ExternalInput")
    qb1T = nc.dram_tensor("qb1T", [H, B], F32, kind="ExternalInput")
    w2rep = nc.dram_tensor("w2rep", [H, 32], BF16, kind="ExternalInput")
    dec_out = nc.dram_tensor("dec_out", [B, VC], F32, kind="ExternalOutput")
    rep_out = nc.dram_tensor("rep_out", [B, TCC], F32, kind="ExternalOutput")

    with tile.TileContext(nc) as tc:
        with (
            tc.tile_pool(name="singles", bufs=1) as singles,
            tc.tile_pool(name="relu", bufs=8) as relu_pool,
            tc.tile_pool(name="stage", bufs=4) as stage_pool,
            tc.tile_pool(name="psA", bufs=2, space="PSUM") as psA,
            tc.tile_pool(name="psB", bufs=2, space="PSUM") as psB,
        ):
          for _rep in range(n_reps):
            w1tT_sb, qb1_sb, w2_sb, decT_sb, tgtT_sb, d_sb, t_sb = (
                [], [], [], [], [], [], [])
            w18_sb = []
            out_stages = []
            # critical-path loads first: DoubleRow weights + first v-chunk
            # of both dec8 tiles so d_proj can start after ~1MB of DMA.
            for i in range(2):
                w18 = singles.tile([128, 2, H], F8, name=f"w18{i}",
                                   tag=f"w18{i}")
                nc.sync.dma_start(out=w18[:, :, :], in_=w1tT8[i])
                w18_sb.append(w18)
                d8 = singles.tile([128, 2, VCP], F8, name=f"dec8{i}",
                                  tag=f"dec8{i}", bufs=2)
                decT_sb.append(d8)
            for dq in range(NSL):
                for i in range(2):
                    nc.sync.dma_start(
                        out=decT_sb[i][:, :, dq * VSL:(dq + 1) * VSL],
                        in_=decT8[i][:, :, dq * VSL:(dq + 1) * VSL])
            for i in range(KC):
                sl_ = slice(i * 128, (i + 1) * 128)
                wt = singles.tile([128, H], BF16, name=f"w1tT{i}", tag=f"w1tT{i}")
                nc.sync.dma_start(out=wt[:, :], in_=w1tT[sl_, :])
                w1tT_sb.append(wt)
                qt = singles.tile([128, B], F32, name=f"qb1{i}", tag=f"qb1{i}")
                nc.sync.dma_start(out=qt[:, :], in_=qb1T[sl_, :])
                qb1_sb.append(qt)
                w2t = singles.tile([128, 32], BF16, name=f"w2{i}", tag=f"w2{i}")
                nc.sync.dma_start(out=w2t[:, :], in_=w2rep[sl_, :])
                w2_sb.append(w2t)
                tg = singles.tile([128, BT], BF16, name=f"tgtT{i}",
                                  tag=f"tgtT{i}", bufs=2)
                for dq in range(2):
                    nc.sync.dma_start(
                        out=tg[:, dq * (BT // 2):(dq + 1) * (BT // 2)],
                        in_=tgtT[sl_, dq * (BT // 2):(dq + 1) * (BT // 2)])
                tgtT_sb.append(tg)
                d_sb.append(
                    singles.tile([128, VCP], BF16, name=f"dsb{i}", tag=f"dsb{i}"))
                if i < 2:
                    out_stage_i = singles.tile([128, VCP], F32,
                                               name=f"outstage{i}",
                                               tag=f"outstage{i}")
                    out_stages.append(out_stage_i)
                t_sb.append(
                    singles.tile([128, BT], BF16, name=f"tsb{i}", tag=f"tsb{i}"))

            # ---- projections: d_projT[k, v] = sum_h w1tT[h,k] * decT[h,v]
            # One [128, VSL] psum tile (2 banks) per (slab, kc); each 512-col
            # half is its own matmul accumulation group; ONE ACT copy per slab
            # so the relu op downstream has a single producer.
            for sl in range(NSL):
                for kc in range(KC):
                    ps = psA.tile([128, VSL], F32, name="proj", tag="proj")
                    slw = min(VSL, VC - sl * VSL)
                    for cp in range(VSL // CW):
                        cpw = min(CW, slw - cp * CW)
                        if cpw <= 0:
                            continue
                        for s2 in range(2):
                            nc.tensor.matmul(
                                ps[:, cp * CW:cp * CW + cpw],
                                lhsT=w18_sb[s2][:, :,
                                                kc * 128:(kc + 1) * 128],
                                rhs=decT_sb[s2][:, :,
                                               sl * VSL + cp * CW:
                                               sl * VSL + cp * CW + cpw],
                                start=(s2 == 0),
                                stop=(s2 == 1),
                                perf_mode=mybir.MatmulPerfMode.DoubleRow,
                            )
                    nc.scalar.mul(
                        out=d_sb[kc][:, sl * VSL:sl * VSL + slw],
                        in_=ps[:, :slw],
                        mul=1.0 / (FP8_SCALE * FP8_SCALE))

            # ---- t_projT[k, (b,t)] = sum_h w1tT[h,k] * tgtT[h,(b,t)]
            for sl in range(BT // VSL):
                for kc in range(KC):
                    ps = psA.tile([128, VSL], F32, name="proj", tag="proj")
                    for cp in range(VSL // CW):
                        for hc in range(KC):
                            nc.tensor.matmul(
                                ps[:, cp * CW:(cp + 1) * CW],
                                lhsT=w1tT_sb[hc][:, kc * 128:(kc + 1) * 128],
                                rhs=tgtT_sb[hc][:,
                                               sl * VSL + cp * CW:
                                               sl * VSL + (cp + 1) * CW],
                                start=(hc == 0),
                                stop=(hc == KC - 1),
                            )
                    nc.scalar.copy(
                        out=t_sb[kc][:, sl * VSL:(sl + 1) * VSL], in_=ps[:, :])

            # ---- repair branch
            for half in range(2):
                psr = psB.tile([128, TCC], F32, name="mvr", tag="mv")
                for kc in range(KC):
                    rts = []
                    for j in range(4):
                        b = half * 4 + j
                        rr = relu_pool.tile([128, TCC], BF16, name="rr", tag="rr")
                        nc.vector.tensor_scalar(
                            out=rr[:, :],
                            in0=t_sb[kc][:, b * TCC:(b + 1) * TCC],
                            scalar1=qb1_sb[kc][:, b:b + 1],
                            scalar2=0.0,
                            op0=AOP.add,
                            op1=AOP.max,
                        )
                        rts.append(rr)
                    for j in range(4):
                        nc.tensor.matmul(
                            psr[32 * j:32 * j + 32, :],
                            lhsT=w2_sb[kc][:, :],
                            rhs=rts[j][:, :],
                            start=(kc == 0),
                            stop=(kc == KC - 1),
                            tile_position=(0, 32 * j),
                        )
                st = stage_pool.tile([128, TCC], F32, name="str", tag="st")
                nc.scalar.copy(out=st[:, :], in_=psr[:, :])
                nc.sync.dma_start(
                    out=rep_out[half * 4:half * 4 + 4, :], in_=st[0:128:32, :])


            # ---- decoder: relu(d_projT + qb1[b]) then matvec with w2
            # col group j <- batch row (half*4 + j); psum rows 32j..32j+31
            # all hold the same result (32x replicated stationary); out-DMA
            # reads rows {0,32,64,96}.
            for sl in range(NSL):
                for half in range(2):
                    mvt = psB.tile([128, VSL], F32, name="mv", tag="mv")
                    mv = [mvt[:, q * CW:(q + 1) * CW]
                          for q in range(VSL // CW)]
                    slw = min(VSL, VC - sl * VSL)  # 1024, last slab 928
                    for kc in range(KC):
                        rts = []
                        for j in range(4):
                            b = half * 4 + j
                            r = relu_pool.tile([128, VSL], BF16, name="r", tag="r")
                            nc.vector.tensor_scalar(
                                out=r[:, :slw],
                                in0=d_sb[kc][:, sl * VSL:sl * VSL + slw],
                                scalar1=qb1_sb[kc][:, b:b + 1],
                                scalar2=0.0,
                                op0=AOP.add,
                                op1=AOP.max,
                            )
                            rts.append(r)
                        for q in range(VSL // CW):
                            qw = min(CW, slw - q * CW)
                            for j in range(4):
                                nc.tensor.matmul(
                                    mv[q][32 * j:32 * j + 32, :qw],
                                    lhsT=w2_sb[kc][:, :],
                                    rhs=rts[j][:, q * CW:q * CW + qw],
                                    start=(kc == 0),
                                    stop=(kc == KC - 1),
                                    tile_position=(0, 32 * j),
                                )
                    base = sl * VSL
                    nc.scalar.copy(
                        out=out_stages[half][0:97, base:base + slw],
                        in_=mvt[0:97, :slw])
                    nc.sync.dma_start(
                        out=dec_out[half * 4:half * 4 + 4, base:base + slw],
                        in_=out_stages[half][0:97:32, base:base + slw])

    nc.compile()
    return nc


def _get_nc(n_reps: int = 1):
    key = f"nc{n_reps}"
    if key not in _cache:
        _cache[key] = _build_nc(n_reps)
    return _cache[key]


def prepare_in_maps(inputs) -> list:
    ie = np.asarray(inputs["input_embeds"], dtype=np.float32)
    te = np.asarray(inputs["target_embeds"], dtype=np.float32)
    mask = np.asarray(inputs["input_mask"], dtype=np.float32)
    w1 = np.asarray(inputs["w1"], dtype=np.float32)
    b1 = np.asarray(inputs["b1"], dtype=np.float32)
    w2 = np.asarray(inputs["w2"], dtype=np.float32)
    dw = np.asarray(inputs["decoder_weight"], dtype=np.float32)

    qb1 = ie[:, 0, :] @ w1[:, :H].T + b1[None, :]  # [B, H] fp32 (exact)
    qb1T = np.ascontiguousarray(qb1.T)  # [H, B]
    w1tT = np.ascontiguousarray(w1[:, H:].T).astype(BF16NP)  # [H, H] (h, k)
    w2rep = np.ascontiguousarray(
        np.broadcast_to(w2[:, None], (H, 32))).astype(BF16NP)
    decT_s = dw.T.astype(np.float32) * FP8_SCALE  # [H, V], scaled for fp8
    # [s, p, i, k] DoubleRow interleave of w1tT * SCALE
    w1tT8 = np.ascontiguousarray(
        (w1[:, H:].T * FP8_SCALE).reshape(2, 2, 128, H)
        .transpose(0, 2, 1, 3)).astype(F8NP)

    in_maps = []
    for c in range(NCORES):
        dshard = np.zeros((H, VCP), dtype=np.float32)
        dshard[:, :VC] = decT_s[:, c * VC:(c + 1) * VC]
        dec8 = np.ascontiguousarray(
            dshard.reshape(2, 2, 128, VCP).transpose(0, 2, 1, 3)).astype(F8NP)
        tgt_sh = te[:, c * TCC:(c + 1) * TCC, :].reshape(BT, H)
        in_maps.append({
            "decT8": dec8,
            "w1tT8": w1tT8,
            "tgtT": np.ascontiguousarray(tgt_sh.T.astype(BF16NP)),
            "w1tT": w1tT,
            "qb1T": qb1T,
            "w2rep": w2rep,
        })
    return in_maps


def kernel(**inputs) -> np.ndarray:
    global last_results
    mask = np.asarray(inputs["input_mask"], dtype=np.float32)
    in_maps = prepare_in_maps(inputs)
    nc = _get_nc()
    res = run_bass_kernel_spmd(
        nc,
        in_maps,
        core_ids=list(range(NCORES)),
        trace=bool(os.environ.get("KERNEL_TRACE")),
    )
    last_results = res

    dec = np.concatenate([res.results[c]["dec_out"] for c in range(NCORES)],
                         axis=1)  # [B, V]
    rep = np.concatenate([res.results[c]["rep_out"] for c in range(NCORES)],
                         axis=1)  # [B, T]
    rep = mask * rep - 1000.0 * (1.0 - mask)
    return np.concatenate([dec, rep], axis=1).astype(np.float32)
